# revision 23
# baseline (speedup 1.0000x reference)
"""GRU-decoder kernel for 8 Trainium2 NeuronCores (v5, collective-free).

Math (all 127 output steps are identical -- see the reference):
    x0   = relu(emb[input[:,0]])                       [B,H]
    h0   = einsum('blh,l->bh', hidden, bridge_w) + bb  [B,H]
    gi   = x0 @ w_ih.T + b_ih ; gh = h0 @ w_hh.T + b_hh
    r,z  = sigmoid(...) ; n = tanh(in + r*hn)
    h1   = (1-z)*n + z*h0
    logp = log_softmax(h1 @ proj_w.T + proj_b)         [B,V]
    out  = broadcast(logp, [B, L-1, V])

v5 strategy: the 8 PJRT launches start with 25-90us of skew, so any
collective forces every core to wait for the slowest one. Instead the
GRU (tiny vs the projection) is computed REDUNDANTLY on every core --
zero collectives, zero sync -- and only the projection + exp-sum are
vocab-sharded. Host combines per-core exp-sums into the global
log-softmax normalizer.

  - bridge: computed in B layout by block-diagonalizing the (l,b)
    contraction -- stationary chunk b' holds bridge_w in column b' only,
    `hidden` streams through as the moving operand, and the 16 chunks
    accumulate in one PSUM group; bridge_b rides in as a K=1 ones-row.
  - h0/h1 transposed to T layout via matmuls against 16*I (the fp8
    scale rides along for free).
  - gates: full 3H rows on every core, fp8 DoubleRow matmuls
    (w x256, x0/h0 x16 -> pre-activations x4096, descaled inside the
    sigmoid/tanh activation `scale`); biases folded in as K=1 rows.
  - projection: vocab-sharded fp8 DoubleRow (proj_w x2048, h1 x16),
    proj_b via bf16 K=1 rows (x32768, issued as the first PSUM pass);
    exp-sum uses activation scale 2^-15; host descales the logits.
"""

import numpy as np
import ml_dtypes

import concourse.bass as bass
import concourse.tile as tile
from concourse import bacc, mybir
from concourse.bass_utils import run_bass_kernel_spmd

B, L, H, V = 16, 128, 1024, 50257
NC = 8
VC = 6656                # per-core vocab shard; 8*VC = 53248 >= V
KC = 8                   # contraction chunks of 128 over H
KK = 4                   # DoubleRow pairs of K-chunks
G3F = 3072               # full gate rows
NEG = -1.0e30

PW_S = 2048.0            # proj_w fp8 scale
H1_S = 16.0              # h1 / h0 / x0 fp8 scale
W_S = 256.0              # gate-weight fp8 scale
HID_S = 8.0              # hidden fp8 scale
BW_S = 64.0              # bridge_w fp8 scale
BR_S = HID_S * BW_S      # bridge psum scale (2^9)
GA_S = H1_S * W_S        # gate pre-activation scale (2^12)
LG_S = PW_S * H1_S       # logits scale (2^15)

f32 = mybir.dt.float32
bf16 = mybir.dt.bfloat16
f8 = mybir.dt.float8e4
FX = mybir.ActivationFunctionType
DR = mybir.MatmulPerfMode.DoubleRow

BF = ml_dtypes.bfloat16
F8 = ml_dtypes.float8_e4m3

GROUPS = [(0, 2048), (2048, 4096), (4096, 6144), (6144, 6656)]

LAST_RESULT = None  # test harness reads profiling info from here
_NC_CACHE = None


def _bc(ap, insert_at, step, count):
    new = list(ap.ap)
    new.insert(insert_at, [step, count])
    return bass.AP(tensor=ap.tensor, offset=ap.offset, ap=new)


def _build():
    nc = bacc.Bacc("TRN2", target_bir_lowering=False, debug=False, num_devices=NC)

    hid = nc.dram_tensor("hid", [L, B, H], f8, kind="ExternalInput").ap()
    bwd = nc.dram_tensor("bwd", [L, B, B], f8, kind="ExternalInput").ap()
    bbrow = nc.dram_tensor("bbrow", [1, H], bf16, kind="ExternalInput").ap()
    x0f = nc.dram_tensor("x0f", [128, KC, B], f8, kind="ExternalInput").ap()
    id16s = nc.dram_tensor("id16s", [B, B], f32, kind="ExternalInput").ap()
    ones1 = nc.dram_tensor("ones1", [1, B], bf16, kind="ExternalInput").ap()
    brow = nc.dram_tensor("brow", [1, 4096], bf16, kind="ExternalInput").ap()
    wih = nc.dram_tensor("wih", [128, KK, 2, G3F], f8, kind="ExternalInput").ap()
    whh = nc.dram_tensor("whh", [128, KK, 2, G3F], f8, kind="ExternalInput").ap()
    pwT = nc.dram_tensor("pwT", [KK, 128, 2, VC], f8, kind="ExternalInput").ap()
    pb = nc.dram_tensor("pb", [1, VC], bf16, kind="ExternalInput").ap()
    logits = nc.dram_tensor("logits", [B, VC], bf16, kind="ExternalOutput").ap()

    with tile.TileContext(nc) as tc:
        with (
            tc.tile_pool(name="singles", bufs=1) as singles,
            tc.tile_pool(name="dram", bufs=1, space="DRAM") as dram,
        ):
            # ---- DMA order = consumption order ---------------------------
            bwd_sb = singles.tile([L, B, B], f8, tag="bwd_sb")
            nc.sync.dma_start(out=bwd_sb, in_=bwd)
            bbrow_sb = singles.tile([1, H], bf16, tag="bbrow_sb")
            nc.sync.dma_start(out=bbrow_sb, in_=bbrow)
            x0f_sb = singles.tile([128, KC, B], f8, tag="x0f_sb")
            nc.sync.dma_start(out=x0f_sb, in_=x0f)
            id16_sb = singles.tile([B, B], f32, tag="id16_sb")
            nc.sync.dma_start(out=id16_sb, in_=id16s)
            ones_sb = singles.tile([1, B], bf16, tag="ones_sb")
            nc.sync.dma_start(out=ones_sb, in_=ones1)
            brow_sb = singles.tile([1, 4096], bf16, tag="brow_sb")
            nc.sync.dma_start(out=brow_sb, in_=brow)
            pb_sb = singles.tile([1, VC], bf16, tag="pb_sb")
            nc.sync.dma_start(out=pb_sb, in_=pb)
            hid_sb = singles.tile([L, B, H], f8, tag="hid_sb")
            nc.sync.dma_start(out=hid_sb, in_=hid)
            wih_sb = singles.tile([128, KK, 2, G3F], f8, tag="wih_sb")
            nc.sync.dma_start(out=wih_sb, in_=wih)
            whh_sb = singles.tile([128, KK, 2, G3F], f8, tag="whh_sb")
            nc.sync.dma_start(out=whh_sb, in_=whh)
            pw_sb = []
            for k in range(KK):
                t = singles.tile([128, 2, VC], f8, tag=f"pw{k}", name=f"pw{k}")
                nc.sync.dma_start(out=t, in_=pwT[k])
                pw_sb.append(t)

            logits_sb = singles.tile([B, VC], bf16, tag="logits_sb")

            h0B_sb = singles.tile([B, H], f32, tag="h0B_sb")
            h0f8 = singles.tile([128, KC, B], f8, tag="h0f8")
            h1f8 = singles.tile([128, KC, B], f8, tag="h1f8")
            trz = singles.tile([B, 2 * H], f32, tag="trz")
            tn = singles.tile([B, H], f32, tag="tn")
            td = singles.tile([B, H], f32, tag="td")

            # ---- bridge, B-layout: stationary chunk b' is bridge_w in
            # column b' only (block-diagonal over the (l,b) contraction);
            # bias rides in as a K=1 ones-row
            with tc.tile_pool(name="br_ps", bufs=1, space="PSUM") as bps:
                h0B_ps = bps.tile([B, H], f32, tag="h0B_ps")
                for hf in range(2):
                    nc.tensor.matmul(
                        h0B_ps[:, hf * 512 : hf * 512 + 512], ones_sb[:],
                        bbrow_sb[0:1, hf * 512 : hf * 512 + 512],
                        start=True, stop=False,
                    )
                for b in range(B):
                    for hf in range(2):
                        nc.tensor.matmul(
                            h0B_ps[:, hf * 512 : hf * 512 + 512],
                            bwd_sb[:, b, :],
                            hid_sb[:, b, hf * 512 : hf * 512 + 512],
                            start=False, stop=(b == B - 1),
                        )
                nc.vector.tensor_scalar_mul(h0B_sb[:], h0B_ps[:], 1.0 / BR_S)

                # h0 -> T layout (x16 via id16s) -> fp8
                h0T_ps = bps.tile([128, KC, B], f32, tag="h0T_ps")
                for hc in range(KC):
                    nc.tensor.matmul(
                        h0T_ps[:, hc, :], h0B_sb[:, hc * 128 : hc * 128 + 128],
                        id16_sb[:], start=True, stop=True,
                    )
                nc.vector.tensor_copy(h0f8[:], h0T_ps[:])

            # ---- gates (full width, redundant on every core) -------------
            with tc.tile_pool(name="g_ps", bufs=1, space="PSUM") as gps:
                grz_ps = gps.tile([B, 2 * H], f32, tag="grz_ps")
                gin_ps = gps.tile([B, H], f32, tag="gin_ps")
                ghn_ps = gps.tile([B, H], f32, tag="ghn_ps")
                # bias rows first (bf16, x4096)
                for so in range(0, 2 * H, 512):
                    nc.tensor.matmul(
                        grz_ps[:, so : so + 512], ones_sb[:],
                        brow_sb[0:1, so : so + 512], start=True, stop=False,
                    )
                for so in range(0, H, 512):
                    nc.tensor.matmul(
                        gin_ps[:, so : so + 512], ones_sb[:],
                        brow_sb[0:1, 2 * H + so : 2 * H + so + 512],
                        start=True, stop=False,
                    )
                    nc.tensor.matmul(
                        ghn_ps[:, so : so + 512], ones_sb[:],
                        brow_sb[0:1, 3 * H + so : 3 * H + so + 512],
                        start=True, stop=False,
                    )
                for kk in range(KK):
                    last = kk == KK - 1
                    for so in range(0, 2 * H, 512):
                        nc.tensor.matmul(
                            grz_ps[:, so : so + 512],
                            x0f_sb[:, 2 * kk : 2 * kk + 2, :],
                            wih_sb[:, kk, :, so : so + 512],
                            start=False, stop=False, perf_mode=DR,
                        )
                        nc.tensor.matmul(
                            grz_ps[:, so : so + 512],
                            h0f8[:, 2 * kk : 2 * kk + 2, :],
                            whh_sb[:, kk, :, so : so + 512],
                            start=False, stop=(last and so == 2 * H - 512),
                            perf_mode=DR,
                        )
                    for so in range(0, H, 512):
                        nc.tensor.matmul(
                            gin_ps[:, so : so + 512],
                            x0f_sb[:, 2 * kk : 2 * kk + 2, :],
                            wih_sb[:, kk, :, 2 * H + so : 2 * H + so + 512],
                            start=False, stop=(last and so == H - 512),
                            perf_mode=DR,
                        )
                        nc.tensor.matmul(
                            ghn_ps[:, so : so + 512],
                            h0f8[:, 2 * kk : 2 * kk + 2, :],
                            whh_sb[:, kk, :, 2 * H + so : 2 * H + so + 512],
                            start=False, stop=(last and so == H - 512),
                            perf_mode=DR,
                        )

                # r,z = sigmoid(grz * 2^-12); n = tanh((gin + r*ghn) * 2^-12)
                nc.scalar.activation(out=trz[:], in_=grz_ps[:], func=FX.Sigmoid,
                                     scale=1.0 / GA_S)
                nc.vector.tensor_mul(tn[:], ghn_ps[:], trz[:, 0:H])
                nc.vector.tensor_add(tn[:], tn[:], gin_ps[:])
                nc.scalar.activation(out=tn[:], in_=tn[:], func=FX.Tanh,
                                     scale=1.0 / GA_S)
                # h1 = n + z * (h0 - n)
                nc.vector.tensor_sub(td[:], h0B_sb[:], tn[:])
                nc.vector.tensor_mul(td[:], td[:], trz[:, H : 2 * H])
                nc.vector.tensor_add(td[:], td[:], tn[:])

            # h1 -> T layout (x16) -> fp8
            with tc.tile_pool(name="h1_ps", bufs=1, space="PSUM") as hps:
                h1T_ps = hps.tile([128, KC, B], f32, tag="h1T_ps")
                for hc in range(KC):
                    nc.tensor.matmul(
                        h1T_ps[:, hc, :], td[:, hc * 128 : hc * 128 + 128],
                        id16_sb[:], start=True, stop=True,
                    )
                nc.vector.tensor_copy(h1f8[:], h1T_ps[:])

            # ---- projection (fp8 DoubleRow) + exp-sum --------------------
            with tc.tile_pool(name="proj_ps", bufs=2, space="PSUM") as pps:
                for gidx, (g0, g1) in enumerate(GROUPS):
                    gw = g1 - g0
                    lg = pps.tile([B, 2048], f32, tag="lg", name="lg")
                    for so in range(0, gw, 512):
                        col = g0 + so
                        nc.tensor.matmul(
                            lg[:, so : so + 512], ones_sb[:],
                            pb_sb[0:1, col : col + 512], start=True, stop=False,
                        )
                    for kk in range(KK):
                        for so in range(0, gw, 512):
                            col = g0 + so
                            nc.tensor.matmul(
                                lg[:, so : so + 512],
                                h1f8[:, 2 * kk : 2 * kk + 2, :],
                                pw_sb[kk][:, :, col : col + 512],
                                start=False, stop=(kk == KK - 1),
                                perf_mode=DR,
                            )
                    nc.vector.tensor_copy(logits_sb[:, g0:g1], lg[:, :gw])
                    nc.sync.dma_start(
                        out=logits[:, g0:g1], in_=logits_sb[:, g0:g1]
                    )

    nc.compile()
    return nc


def kernel(input, hidden, emb, bridge_w, bridge_b, w_ih, w_hh, b_ih, b_hh,
           proj_w, proj_b):
    global _NC_CACHE, LAST_RESULT
    if _NC_CACHE is None:
        _NC_CACHE = _build()
    nc = _NC_CACHE

    input = np.asarray(input)
    hidden = np.asarray(hidden, dtype=np.float32)
    emb = np.asarray(emb, dtype=np.float32)
    bridge_w = np.asarray(bridge_w, dtype=np.float32)
    bridge_b = np.asarray(bridge_b, dtype=np.float32)
    w_ih = np.asarray(w_ih, dtype=np.float32)
    w_hh = np.asarray(w_hh, dtype=np.float32)
    b_ih = np.asarray(b_ih, dtype=np.float32)
    b_hh = np.asarray(b_hh, dtype=np.float32)
    proj_w = np.asarray(proj_w, dtype=np.float32)
    proj_b = np.asarray(proj_b, dtype=np.float32)

    x0 = np.maximum(emb[input[:, 0].astype(np.int64)], 0.0)   # [B, H] relu
    x0f_in = np.ascontiguousarray(
        (x0.T * H1_S).reshape(KC, 128, B).transpose(1, 0, 2).astype(F8))
    hid_in = np.ascontiguousarray((hidden.transpose(1, 0, 2) * HID_S).astype(F8))
    bwd_np = np.zeros((L, B, B), np.float32)
    for b_ in range(B):
        bwd_np[:, b_, b_] = bridge_w.reshape(L) * BW_S
    bwd_in = np.ascontiguousarray(bwd_np.astype(F8))
    bbrow_in = np.ascontiguousarray(
        np.full((1, H), float(bridge_b.reshape(-1)[0]) * BR_S, np.float32).astype(BF))
    ones_in = np.ones((1, B), dtype=BF)
    id16_in = np.ascontiguousarray((np.eye(B) * H1_S).astype(np.float32))
    brow_in = np.ascontiguousarray((np.concatenate([
        (b_ih + b_hh)[: 2 * H], b_ih[2 * H :], b_hh[2 * H :],
    ]) * GA_S).reshape(1, 4096).astype(BF))

    def pack_w(w):  # [3H, H] -> [128, KK, 2, 3H] fp8, x W_S
        wT = (w.T * W_S).reshape(KK, 2, 128, G3F)     # [kk, i, p, j]
        return np.ascontiguousarray(wT.transpose(2, 0, 1, 3).astype(F8))

    wih_in = pack_w(w_ih)
    whh_in = pack_w(w_hh)

    in_maps = []
    for c in range(NC):
        lo, hi = c * VC, min((c + 1) * VC, V)
        pw_blk = proj_w[lo:hi]
        pb_blk = proj_b[lo:hi]
        if hi - lo < VC:
            pad = VC - (hi - lo)
            pw_blk = np.concatenate(
                [pw_blk, np.zeros((pad, H), np.float32)], axis=0)
            pb_blk = np.concatenate([pb_blk, np.full((pad,), NEG, np.float32)])
        in_maps.append({
            "hid": hid_in,
            "bwd": bwd_in,
            "bbrow": bbrow_in,
            "x0f": x0f_in,
            "id16s": id16_in,
            "ones1": ones_in,
            "brow": brow_in,
            "wih": wih_in,
            "whh": whh_in,
            "pwT": np.ascontiguousarray(
                (pw_blk.T * PW_S).reshape(KK, 2, 128, VC)
                .transpose(0, 2, 1, 3).astype(F8)),
            "pb": np.ascontiguousarray(
                (pb_blk * LG_S).reshape(1, VC).astype(BF)),
        })

    res = run_bass_kernel_spmd(nc, in_maps, list(range(NC)))
    LAST_RESULT = res

    logits_full = np.concatenate(
        [res.results[c]["logits"].astype(np.float32) for c in range(NC)], axis=1
    )[:, :V] * (1.0 / LG_S)
    lse = np.log(np.exp(logits_full.astype(np.float64)).sum(axis=1)
                 ).astype(np.float32)                 # [B]
    logp = np.ascontiguousarray(logits_full - lse[:, None])
    return np.broadcast_to(logp[:, None, :], (B, L - 1, V))


# revision 25
# speedup vs baseline: 1.0987x; 1.0987x over previous
"""GRU-decoder kernel for 8 Trainium2 NeuronCores (v5, collective-free).

Math (all 127 output steps are identical -- see the reference):
    x0   = relu(emb[input[:,0]])                       [B,H]
    h0   = einsum('blh,l->bh', hidden, bridge_w) + bb  [B,H]
    gi   = x0 @ w_ih.T + b_ih ; gh = h0 @ w_hh.T + b_hh
    r,z  = sigmoid(...) ; n = tanh(in + r*hn)
    h1   = (1-z)*n + z*h0
    logp = log_softmax(h1 @ proj_w.T + proj_b)         [B,V]
    out  = broadcast(logp, [B, L-1, V])

v5 strategy: the 8 PJRT launches start with 25-90us of skew, so any
collective forces every core to wait for the slowest one. Instead the
GRU (tiny vs the projection) is computed REDUNDANTLY on every core --
zero collectives, zero sync -- and only the projection is
vocab-sharded. The host computes the global log-softmax normalizer
directly from the gathered logits.

  - bridge: computed in B layout by block-diagonalizing the (l,b)
    contraction -- stationary chunk b' holds bridge_w in column b' only,
    `hidden` streams through as the moving operand, and the 16 chunks
    accumulate in one PSUM group; bridge_b rides in as a K=1 ones-row.
  - h0/h1 transposed to T layout via matmuls against 16*I (the fp8
    scale rides along for free).
  - gates: full 3H rows on every core, fp8 DoubleRow matmuls
    (w x256, x0/h0 x16 -> pre-activations x4096, descaled inside the
    sigmoid/tanh activation `scale`); biases folded in as K=1 rows.
  - projection: vocab-sharded fp8 DoubleRow (proj_w x2048, h1 x16),
    proj_b via bf16 K=1 rows (x32768, issued as the first PSUM pass so
    they run before h1 is ready); host descales the logits by 2^-15.
"""

import numpy as np
import ml_dtypes

import concourse.bass as bass
import concourse.tile as tile
from concourse import bacc, mybir
from concourse.bass_utils import run_bass_kernel_spmd

B, L, H, V = 16, 128, 1024, 50257
NC = 8
VC = 6656                # per-core vocab shard; 8*VC = 53248 >= V
KC = 8                   # contraction chunks of 128 over H
KK = 4                   # DoubleRow pairs of K-chunks
G3F = 3072               # full gate rows
NEG = -1.0e30

PW_S = 2048.0            # proj_w fp8 scale
H1_S = 16.0              # h1 / h0 / x0 fp8 scale
W_S = 256.0              # gate-weight fp8 scale
HID_S = 8.0              # hidden fp8 scale
BW_S = 64.0              # bridge_w fp8 scale
BR_S = HID_S * BW_S      # bridge psum scale (2^9)
GA_S = H1_S * W_S        # gate pre-activation scale (2^12)
LG_S = PW_S * H1_S       # logits scale (2^15)

f32 = mybir.dt.float32
bf16 = mybir.dt.bfloat16
f8 = mybir.dt.float8e4
FX = mybir.ActivationFunctionType
DR = mybir.MatmulPerfMode.DoubleRow

BF = ml_dtypes.bfloat16
F8 = ml_dtypes.float8_e4m3

GROUPS = [(0, 2048), (2048, 4096), (4096, 6144), (6144, 6656)]

LAST_RESULT = None  # test harness reads profiling info from here
_NC_CACHE = None


def _bc(ap, insert_at, step, count):
    new = list(ap.ap)
    new.insert(insert_at, [step, count])
    return bass.AP(tensor=ap.tensor, offset=ap.offset, ap=new)


def _build():
    nc = bacc.Bacc("TRN2", target_bir_lowering=False, debug=False, num_devices=NC)

    hid = nc.dram_tensor("hid", [L, B, H], f8, kind="ExternalInput").ap()
    bwd = nc.dram_tensor("bwd", [L, B, B], f8, kind="ExternalInput").ap()
    bbrow = nc.dram_tensor("bbrow", [1, H], bf16, kind="ExternalInput").ap()
    x0f = nc.dram_tensor("x0f", [128, KC, B], f8, kind="ExternalInput").ap()
    id16s = nc.dram_tensor("id16s", [B, B], f32, kind="ExternalInput").ap()
    ones1 = nc.dram_tensor("ones1", [1, B], bf16, kind="ExternalInput").ap()
    brow = nc.dram_tensor("brow", [1, 4096], bf16, kind="ExternalInput").ap()
    wih = nc.dram_tensor("wih", [128, KK, 2, G3F], f8, kind="ExternalInput").ap()
    whh = nc.dram_tensor("whh", [128, KK, 2, G3F], f8, kind="ExternalInput").ap()
    pwT = nc.dram_tensor("pwT", [KK, 128, 2, VC], f8, kind="ExternalInput").ap()
    pb = nc.dram_tensor("pb", [1, VC], f32, kind="ExternalInput").ap()
    logits = nc.dram_tensor("logits", [B, VC], bf16, kind="ExternalOutput").ap()

    with tile.TileContext(nc) as tc:
        with (
            tc.tile_pool(name="singles", bufs=1) as singles,
            tc.tile_pool(name="dram", bufs=1, space="DRAM") as dram,
        ):
            # ---- DMA order = consumption order ---------------------------
            bwd_sb = singles.tile([L, B, B], f8, tag="bwd_sb")
            nc.sync.dma_start(out=bwd_sb, in_=bwd)
            bbrow_sb = singles.tile([1, H], bf16, tag="bbrow_sb")
            nc.sync.dma_start(out=bbrow_sb, in_=bbrow)
            x0f_sb = singles.tile([128, KC, B], f8, tag="x0f_sb")
            nc.sync.dma_start(out=x0f_sb, in_=x0f)
            id16_sb = singles.tile([B, B], f32, tag="id16_sb")
            nc.sync.dma_start(out=id16_sb, in_=id16s)
            ones_sb = singles.tile([1, B], bf16, tag="ones_sb")
            nc.sync.dma_start(out=ones_sb, in_=ones1)
            brow_sb = singles.tile([1, 4096], bf16, tag="brow_sb")
            nc.sync.dma_start(out=brow_sb, in_=brow)
            hid_sb = singles.tile([L, B, H], f8, tag="hid_sb")
            nc.sync.dma_start(out=hid_sb, in_=hid)
            wih_sb = singles.tile([128, KK, 2, G3F], f8, tag="wih_sb")
            nc.sync.dma_start(out=wih_sb, in_=wih)
            whh_sb = singles.tile([128, KK, 2, G3F], f8, tag="whh_sb")
            nc.sync.dma_start(out=whh_sb, in_=whh)
            pw_sb = []
            for k in range(KK):
                t = singles.tile([128, 2, VC], f8, tag=f"pw{k}", name=f"pw{k}")
                nc.sync.dma_start(out=t, in_=pwT[k])
                pw_sb.append(t)
            pbb_sb = singles.tile([B, VC], f32, tag="pbb_sb")
            nc.sync.dma_start(out=pbb_sb, in_=_bc(pb[0], 0, 0, B))

            logits_sb = singles.tile([B, VC], bf16, tag="logits_sb")

            h0B_sb = singles.tile([B, H], f32, tag="h0B_sb")
            h0f8 = singles.tile([128, KC, B], f8, tag="h0f8")
            h1f8 = singles.tile([128, KC, B], f8, tag="h1f8")
            trz = singles.tile([B, 2 * H], f32, tag="trz")
            tn = singles.tile([B, H], f32, tag="tn")
            td = singles.tile([B, H], f32, tag="td")

            # ---- bridge, B-layout: stationary chunk b' is bridge_w in
            # column b' only (block-diagonal over the (l,b) contraction);
            # bias rides in as a K=1 ones-row
            with tc.tile_pool(name="br_ps", bufs=1, space="PSUM") as bps:
                h0B_ps = bps.tile([B, H], f32, tag="h0B_ps")
                for hf in range(2):
                    nc.tensor.matmul(
                        h0B_ps[:, hf * 512 : hf * 512 + 512], ones_sb[:],
                        bbrow_sb[0:1, hf * 512 : hf * 512 + 512],
                        start=True, stop=False,
                    )
                for b in range(B):
                    for hf in range(2):
                        nc.tensor.matmul(
                            h0B_ps[:, hf * 512 : hf * 512 + 512],
                            bwd_sb[:, b, :],
                            hid_sb[:, b, hf * 512 : hf * 512 + 512],
                            start=False, stop=(b == B - 1),
                        )
                nc.vector.tensor_scalar_mul(h0B_sb[:], h0B_ps[:], 1.0 / BR_S)

                # h0 -> T layout (x16 via id16s) -> fp8
                h0T_ps = bps.tile([128, KC, B], f32, tag="h0T_ps")
                for hc in range(KC):
                    nc.tensor.matmul(
                        h0T_ps[:, hc, :], h0B_sb[:, hc * 128 : hc * 128 + 128],
                        id16_sb[:], start=True, stop=True,
                    )
                nc.vector.tensor_copy(h0f8[:], h0T_ps[:])

            # ---- gates (full width, redundant on every core) -------------
            with tc.tile_pool(name="g_ps", bufs=1, space="PSUM") as gps:
                grz_ps = gps.tile([B, 2 * H], f32, tag="grz_ps")
                gin_ps = gps.tile([B, H], f32, tag="gin_ps")
                ghn_ps = gps.tile([B, H], f32, tag="ghn_ps")
                # bias rows first (bf16, x4096)
                for so in range(0, 2 * H, 512):
                    nc.tensor.matmul(
                        grz_ps[:, so : so + 512], ones_sb[:],
                        brow_sb[0:1, so : so + 512], start=True, stop=False,
                    )
                for so in range(0, H, 512):
                    nc.tensor.matmul(
                        gin_ps[:, so : so + 512], ones_sb[:],
                        brow_sb[0:1, 2 * H + so : 2 * H + so + 512],
                        start=True, stop=False,
                    )
                    nc.tensor.matmul(
                        ghn_ps[:, so : so + 512], ones_sb[:],
                        brow_sb[0:1, 3 * H + so : 3 * H + so + 512],
                        start=True, stop=False,
                    )
                for kk in range(KK):
                    last = kk == KK - 1
                    for so in range(0, 2 * H, 512):
                        nc.tensor.matmul(
                            grz_ps[:, so : so + 512],
                            x0f_sb[:, 2 * kk : 2 * kk + 2, :],
                            wih_sb[:, kk, :, so : so + 512],
                            start=False, stop=False, perf_mode=DR,
                        )
                        nc.tensor.matmul(
                            grz_ps[:, so : so + 512],
                            h0f8[:, 2 * kk : 2 * kk + 2, :],
                            whh_sb[:, kk, :, so : so + 512],
                            start=False, stop=(last and so == 2 * H - 512),
                            perf_mode=DR,
                        )
                    for so in range(0, H, 512):
                        nc.tensor.matmul(
                            gin_ps[:, so : so + 512],
                            x0f_sb[:, 2 * kk : 2 * kk + 2, :],
                            wih_sb[:, kk, :, 2 * H + so : 2 * H + so + 512],
                            start=False, stop=(last and so == H - 512),
                            perf_mode=DR,
                        )
                        nc.tensor.matmul(
                            ghn_ps[:, so : so + 512],
                            h0f8[:, 2 * kk : 2 * kk + 2, :],
                            whh_sb[:, kk, :, 2 * H + so : 2 * H + so + 512],
                            start=False, stop=(last and so == H - 512),
                            perf_mode=DR,
                        )

                # r,z = sigmoid(grz * 2^-12); n = tanh((gin + r*ghn) * 2^-12)
                nc.scalar.activation(out=trz[:], in_=grz_ps[:], func=FX.Sigmoid,
                                     scale=1.0 / GA_S)
                nc.vector.tensor_mul(tn[:], ghn_ps[:], trz[:, 0:H])
                nc.vector.tensor_add(tn[:], tn[:], gin_ps[:])
                nc.scalar.activation(out=tn[:], in_=tn[:], func=FX.Tanh,
                                     scale=1.0 / GA_S)
                # h1 = n + z * (h0 - n)
                nc.vector.tensor_sub(td[:], h0B_sb[:], tn[:])
                nc.vector.tensor_mul(td[:], td[:], trz[:, H : 2 * H])
                nc.vector.tensor_add(td[:], td[:], tn[:])

            # h1 -> T layout (x16) -> fp8
            with tc.tile_pool(name="h1_ps", bufs=1, space="PSUM") as hps:
                h1T_ps = hps.tile([128, KC, B], f32, tag="h1T_ps")
                for hc in range(KC):
                    nc.tensor.matmul(
                        h1T_ps[:, hc, :], td[:, hc * 128 : hc * 128 + 128],
                        id16_sb[:], start=True, stop=True,
                    )
                nc.vector.tensor_copy(h1f8[:], h1T_ps[:])

            # ---- projection (fp8 DoubleRow) + exp-sum --------------------
            with tc.tile_pool(name="proj_ps", bufs=2, space="PSUM") as pps:
                for gidx, (g0, g1) in enumerate(GROUPS):
                    gw = g1 - g0
                    lg = pps.tile([B, 2048], f32, tag="lg", name="lg")
                    for kk in range(KK):
                        for so in range(0, gw, 512):
                            col = g0 + so
                            nc.tensor.matmul(
                                lg[:, so : so + 512],
                                h1f8[:, 2 * kk : 2 * kk + 2, :],
                                pw_sb[kk][:, :, col : col + 512],
                                start=(kk == 0), stop=(kk == KK - 1),
                                perf_mode=DR,
                            )
                    # pb folded into the PSUM->SBUF copy (frees 13 PE matmuls)
                    nc.vector.tensor_add(logits_sb[:, g0:g1], lg[:, :gw],
                                         pbb_sb[:, g0:g1])
                    nc.sync.dma_start(
                        out=logits[:, g0:g1], in_=logits_sb[:, g0:g1]
                    )

    nc.compile()
    return nc


def kernel(input, hidden, emb, bridge_w, bridge_b, w_ih, w_hh, b_ih, b_hh,
           proj_w, proj_b):
    global _NC_CACHE, LAST_RESULT
    if _NC_CACHE is None:
        _NC_CACHE = _build()
    nc = _NC_CACHE

    input = np.asarray(input)
    hidden = np.asarray(hidden, dtype=np.float32)
    emb = np.asarray(emb, dtype=np.float32)
    bridge_w = np.asarray(bridge_w, dtype=np.float32)
    bridge_b = np.asarray(bridge_b, dtype=np.float32)
    w_ih = np.asarray(w_ih, dtype=np.float32)
    w_hh = np.asarray(w_hh, dtype=np.float32)
    b_ih = np.asarray(b_ih, dtype=np.float32)
    b_hh = np.asarray(b_hh, dtype=np.float32)
    proj_w = np.asarray(proj_w, dtype=np.float32)
    proj_b = np.asarray(proj_b, dtype=np.float32)

    x0 = np.maximum(emb[input[:, 0].astype(np.int64)], 0.0)   # [B, H] relu
    x0f_in = np.ascontiguousarray(
        (x0.T * H1_S).reshape(KC, 128, B).transpose(1, 0, 2).astype(F8))
    hid_in = np.ascontiguousarray((hidden.transpose(1, 0, 2) * HID_S).astype(F8))
    bwd_np = np.zeros((L, B, B), np.float32)
    for b_ in range(B):
        bwd_np[:, b_, b_] = bridge_w.reshape(L) * BW_S
    bwd_in = np.ascontiguousarray(bwd_np.astype(F8))
    bbrow_in = np.ascontiguousarray(
        np.full((1, H), float(bridge_b.reshape(-1)[0]) * BR_S, np.float32).astype(BF))
    ones_in = np.ones((1, B), dtype=BF)
    id16_in = np.ascontiguousarray((np.eye(B) * H1_S).astype(np.float32))
    brow_in = np.ascontiguousarray((np.concatenate([
        (b_ih + b_hh)[: 2 * H], b_ih[2 * H :], b_hh[2 * H :],
    ]) * GA_S).reshape(1, 4096).astype(BF))

    def pack_w(w):  # [3H, H] -> [128, KK, 2, 3H] fp8, x W_S
        wT = (w.T * W_S).reshape(KK, 2, 128, G3F)     # [kk, i, p, j]
        return np.ascontiguousarray(wT.transpose(2, 0, 1, 3).astype(F8))

    wih_in = pack_w(w_ih)
    whh_in = pack_w(w_hh)

    in_maps = []
    for c in range(NC):
        lo, hi = c * VC, min((c + 1) * VC, V)
        pw_blk = proj_w[lo:hi]
        pb_blk = proj_b[lo:hi]
        if hi - lo < VC:
            pad = VC - (hi - lo)
            pw_blk = np.concatenate(
                [pw_blk, np.zeros((pad, H), np.float32)], axis=0)
            pb_blk = np.concatenate([pb_blk, np.full((pad,), NEG, np.float32)])
        in_maps.append({
            "hid": hid_in,
            "bwd": bwd_in,
            "bbrow": bbrow_in,
            "x0f": x0f_in,
            "id16s": id16_in,
            "ones1": ones_in,
            "brow": brow_in,
            "wih": wih_in,
            "whh": whh_in,
            "pwT": np.ascontiguousarray(
                (pw_blk.T * PW_S).reshape(KK, 2, 128, VC)
                .transpose(0, 2, 1, 3).astype(F8)),
            "pb": np.ascontiguousarray(
                (pb_blk * LG_S).reshape(1, VC).astype(np.float32)),
        })

    res = run_bass_kernel_spmd(nc, in_maps, list(range(NC)))
    LAST_RESULT = res

    logits_full = np.concatenate(
        [res.results[c]["logits"].astype(np.float32) for c in range(NC)], axis=1
    )[:, :V] * (1.0 / LG_S)
    lse = np.log(np.exp(logits_full.astype(np.float64)).sum(axis=1)
                 ).astype(np.float32)                 # [B]
    logp = np.ascontiguousarray(logits_full - lse[:, None])
    return np.broadcast_to(logp[:, None, :], (B, L - 1, V))
